# revision 11
# baseline (speedup 1.0000x reference)
"""Trainium2 Bass kernel for an involution Bottleneck block (B=2, Cin=256,
Cmid=64, Cout=256, H=W=56, K=15, G=4).

Sharding: 8 cores = 2 batches x 4 H-quarters (14 output rows each). Each core
receives a zero-padded input halo [256, 28, 70] (7 rows/cols each side), so no
inter-core communication is needed (halo compute is redundant).

Per-core pipeline (channels on SBUF partitions, pixels on free dim):
  conv1 1x1 (PE, bf16) -> BN+ReLU (ACT) -> out1 [64, 28x70] bf16, plus a copy
    shifted by +7 rows at partitions 64:128 and a +1-col copy (odd-kx align).
  reduce 1x1 (PE) -> BN+ReLU (ACT) -> r [16, 784] bf16 (+ ones row for bias).
  involution as 113 tap-PAIRS (ky, ky+7): per pair one span matmul with M=128
    (2 taps x 64 group-expanded channels, BN2 gamma folded in, bias via the
    ones-row at K=17) into a 6-bank PSUM ring (3 pairs in flight).  Products:
      E-pairs: ACT evicts the pair to bf16 SBUF, DVE tensor_mul with the
        shifted out1 window.
      D-pairs (~3/10): DVE multiplies directly from fp32 PSUM (no eviction),
        relieving ACT.
    Accumulation: most product planes are summed ON THE PE as fp32 via
    identity matmuls into a dedicated 2-bank PSUM accumulator (lagged a few
    pairs so the plane is ready); every 8th plane instead goes through a
    small DVE bf16 add tree to keep DVE/PE balanced.
  merge ky-halves + both accumulators (DMA + DVE adds) -> +b2, ReLU (ACT) ->
  conv3 1x1 with g3 folded into W3 (PE), accumulating the bf16 residual via
  an identity matmul -> +b3, ReLU (ACT) -> DMA out.
"""

import sys, types
sys.path.insert(0, "/opt/trn_rl_repo")

import numpy as np
import ml_dtypes
from contextlib import ExitStack

import concourse.bass as bass
import concourse.mybir as mybir
import concourse.tile as tile
from concourse import bacc
from concourse.bass import ts
from concourse.bass_utils import run_bass_kernel_spmd

BF16 = mybir.dt.bfloat16
F32 = mybir.dt.float32
AF = mybir.ActivationFunctionType

K = 15
G = 4
GC = 16
PAD = 7
CIN = 256
CMID = 64
RED = 16
COUT = 256
H = 56
W = 56
B = 2
HB = 14            # output rows per core
HP = HB + 2 * PAD  # 28 padded rows
WP = W + 2 * PAD   # 70 padded cols
NP = HP * WP       # 1960
HH = HB // 2       # 7 rows per half-block
NF = HH * W        # 392 pixels per half-block
NPAIR = 7 * K + 8  # 105 (ky,ky+7) pairs + 8 row-14 (kx,kx+1) pairs = 113
WCH = 38           # wse pairs per 32-partition chunk

IDENT_LAG = 3      # pairs between a plane's mul and its PE identity-add


def _is_direct(pi):
    return pi % 10 in (2, 5, 8)


def _is_tree(pi):
    return pi % 8 == 3


_PROGRAM = None  # (nc, names) cache


def _build_program():
    n_tree = sum(1 for pi in range(NPAIR) if _is_tree(pi))
    nc = bacc.Bacc(None, target_bir_lowering=False, debug=False)
    with tile.TileContext(nc) as tc, ExitStack() as ctx:
        dram = ctx.enter_context(tc.tile_pool(name="dram", bufs=1, space="DRAM"))
        xb_d = dram.tile([CIN, NP], BF16, kind="ExternalInput", name="xb")
        xr_d = dram.tile([COUT, HB * W], BF16, kind="ExternalInput", name="xr")
        w1t_d = dram.tile([CIN, CMID], BF16, kind="ExternalInput", name="w1t")
        wrt_d = dram.tile([CMID, RED], BF16, kind="ExternalInput", name="wrt")
        wse_d = dram.tile([81, WCH * 128], BF16, kind="ExternalInput", name="wse")
        w3t_d = dram.tile([CMID, COUT], BF16, kind="ExternalInput", name="w3t")
        vec_d = dram.tile([128, 7], F32, kind="ExternalInput", name="vecs")
        ones_d = dram.tile([1, 2 * NF], BF16, kind="ExternalInput", name="ones")
        id_d = dram.tile([128, 128], BF16, kind="ExternalInput", name="ident")
        y_d = dram.tile([COUT, HB * W], F32, kind="ExternalOutput", name="y")

        wpool = ctx.enter_context(tc.tile_pool(name="weights", bufs=1))
        w1t = wpool.tile([128, 2, CMID], BF16)
        nc.sync.dma_start(out=w1t[:], in_=w1t_d[:].rearrange("(c p) m -> p c m", p=128))

        # x halo load in 4 chunks matching conv1's j-loop so conv1 can start
        # as soon as its first chunk lands
        xpool = ctx.enter_context(tc.tile_pool(name="xin", bufs=1))
        xb = xpool.tile([128, 2, NP], BF16)
        xb_src = xb_d[:].rearrange("(c p) n -> p c n", p=128)
        for j in (1, 2, 3, 0):
            nc.sync.dma_start(out=xb[:, :, ts(j, 490)], in_=xb_src[:, :, ts(j, 490)])

        wrt = wpool.tile([CMID, RED], BF16)
        nc.sync.dma_start(out=wrt[:], in_=wrt_d[:])
        vecs = wpool.tile([128, 7], F32)
        nc.sync.dma_start(out=vecs[:], in_=vec_d[:])
        ident = wpool.tile([128, 128], BF16)
        nc.sync.dma_start(out=ident[:], in_=id_d[:])
        # span weights in 3 vertical chunks at partitions 0/32/64 so the
        # DMA spans 81 partitions and fans out across the HWDGE queues
        wse = wpool.tile([81, WCH * 128], BF16)
        nc.sync.dma_start(out=wse[:], in_=wse_d[:])
        w3t = wpool.tile([CMID, COUT], BF16)
        nc.sync.dma_start(out=w3t[:], in_=w3t_d[:])
        xrb = xpool.tile([128, 2, HB * W], BF16)
        nc.sync.dma_start(out=xrb[:], in_=xr_d[:].rearrange("(c p) n -> p c n", p=128))

        opool = ctx.enter_context(tc.tile_pool(name="out1", bufs=1))
        out1p = opool.tile([128, NP], BF16)
        out1q = opool.tile([128, NP], BF16)
        out1r = opool.tile([128, NP], BF16)  # row-14 pairs: [0:64]=blockA, [64:128]=blockA<<1col

        # conv1: out1 = relu(g1 * (W1 @ x) + b1) over all 28x70 padded pixels
        with tc.tile_pool(name="p1", bufs=4, space="PSUM") as p1:
            for j in (1, 2, 3, 0):
                ps = p1.tile([CMID, 490], F32, tag="ps1")
                nc.tensor.matmul(ps[:], w1t[:, 0, :], xb[:, 0, ts(j, 490)],
                                 start=True, stop=False)
                nc.tensor.matmul(ps[:], w1t[:, 1, :], xb[:, 1, ts(j, 490)],
                                 start=False, stop=True)
                nc.scalar.activation(out1p[0:CMID, ts(j, 490)], ps[:], AF.Relu,
                                     bias=vecs[0:CMID, 1:2], scale=vecs[0:CMID, 0:1])

        # rows 7..27 duplicated at partitions 64:128 (the +7-row tap shift);
        # zero the unwritten tail there so zero-padded taps read 0, not junk.
        DUPW = (HP - HH) * WP  # 1470
        nc.vector.memset(out1p[CMID:128, DUPW:NP], 0.0)
        nc.vector.memset(out1q[CMID:128, DUPW - 1:NP], 0.0)
        nc.vector.memset(out1q[0:CMID, NP - 1:NP], 0.0)
        nc.sync.dma_start(out=out1p[CMID:128, 0:DUPW],
                          in_=out1p[0:CMID, HH * WP:NP])
        o3 = out1p[:].rearrange("p (h w) -> p h w", w=WP)
        o3q = out1q[:].rearrange("p (h w) -> p h w", w=WP)
        o3r = out1r[:].rearrange("p (h w) -> p h w", w=WP)

        spool = ctx.enter_context(tc.tile_pool(name="stage", bufs=1))
        r_sb = spool.tile([81, 2 * NF], BF16)
        # ones row (span bias via the K dimension); DMA since engines cannot
        # address a single partition at offset 16
        nc.sync.dma_start(out=r_sb[RED:RED + 1, :], in_=ones_d[:])

        # reduce: r = relu(gr * (Wr @ out1_central) + br), central 14x56 pixels
        with tc.tile_pool(name="pr", bufs=2, space="PSUM") as pr:
            for hhalf in range(2):
                ps = pr.tile([RED, NF], F32, tag="psr")
                nc.tensor.matmul(ps[:], wrt[:],
                                 o3[0:CMID, PAD + HH * hhalf:PAD + HH * (hhalf + 1), PAD:PAD + W],
                                 start=True, stop=True)
                nc.scalar.activation(r_sb[0:RED, ts(hhalf, NF)], ps[:], AF.Relu,
                                     bias=vecs[0:RED, 3:4], scale=vecs[0:RED, 2:3])

        # replicate r (+ones row) at partitions 32/64 to match the span
        # lhsT chunks' base partitions
        for c in range(1, 3):
            nc.sync.dma_start(out=r_sb[32 * c:32 * c + RED + 1, :],
                              in_=r_sb[0:RED + 1, :])

        # +1-col shifted copies (odd-kx 4B alignment) are same-partition, so
        # they run on DVE; placed after the reduce so r is produced first
        nc.vector.tensor_copy(out1q[0:CMID, 0:NP - 1], out1p[0:CMID, 1:NP])
        nc.vector.tensor_copy(out1q[CMID:128, 0:DUPW - 1], out1p[CMID:128, 1:DUPW])
        # row-14 pair source: partitions 64:128 hold block A shifted +1 col
        nc.vector.memset(out1r[CMID:128, NP - 1:NP], 0.0)
        nc.vector.tensor_copy(out1r[0:CMID, :], out1p[0:CMID, :])
        nc.sync.dma_start(out=out1r[CMID:128, 0:NP - 1], in_=out1p[0:CMID, 1:NP])

        def window(pi):
            """shifted out1 window for pair pi as a [128, 2, 7, 56] view"""
            if pi < 7 * K:
                ky, kx = pi // K, pi % K
                if kx % 2 == 0:
                    src_ = o3[:, ky:ky + 2 * HH, kx:kx + W]
                else:
                    src_ = o3q[:, ky:ky + 2 * HH, kx - 1:kx - 1 + W]
            else:
                kx = 2 * (pi - 7 * K)  # row-14 pair (14,kx)+(14,kx+1)
                src_ = o3r[:, 14:14 + 2 * HH, kx:kx + W]
            return src_.rearrange("p (b h) w -> p b h w", b=2)

        # involution span + multiply; PE accumulates most product planes as
        # fp32 via identity matmuls, a small DVE bf16 tree takes the rest
        accT = None     # DVE tree accumulator (bf16)
        prod8 = None
        tree_j = 0
        dve_planes = []  # plane APs pending PE identity-add
        first_ident = [True, True]
        accP_sb = spool.tile([128, 2 * NF], BF16)
        with tc.tile_pool(name="sp", bufs=1, space="PSUM") as sp, \
             tc.tile_pool(name="we", bufs=4) as we_pool, \
             tc.tile_pool(name="plp", bufs=6) as plane_pool, \
             tc.tile_pool(name="prod", bufs=2) as prod_pool, \
             tc.tile_pool(name="accp", bufs=2) as acc_pool:
            ring = sp.tile([128, 3072], F32)   # 6 banks: 3 pair-slots
            accp = sp.tile([128, 1024], F32)   # 2 banks: fp32 plane accumulator
            rv = ring[:].rearrange("p (s x) -> p s x", x=512)

            def flush_ident(count):
                while len(dve_planes) > count:
                    pl = dve_planes.pop(0)
                    for h in range(2):
                        nc.tensor.matmul(accp[:, h * 512:h * 512 + NF], ident[:],
                                         pl[:, ts(h, NF)],
                                         start=first_ident[h], stop=False,
                                         skip_group_check=True)
                        first_ident[h] = False

            for pi in range(NPAIR):
                sl = pi % 3
                wc, wo = pi // WCH, pi % WCH
                lhsT = wse[32 * wc:32 * wc + RED + 1, ts(wo, 128)]
                rr = r_sb[32 * wc:32 * wc + RED + 1, :]
                for h in range(2):
                    nc.tensor.matmul(ring[:, (2 * sl + h) * 512:(2 * sl + h) * 512 + NF],
                                     lhsT, rr[:, ts(h, NF)], start=True, stop=True)
                # lagged identity-adds keep PE busy while consumers catch up
                flush_ident(IDENT_LAG)

                to_tree = _is_tree(pi)
                if to_tree:
                    if tree_j % 8 == 0:
                        prod8 = prod_pool.tile([128, 8, 2 * NF], BF16, tag="prod")
                    plane = prod8[:, tree_j % 8, :]
                else:
                    pt = plane_pool.tile([128, 2 * NF], BF16, tag="plane")
                    plane = pt[:]
                pv4 = plane.rearrange("p (b h w) -> p b h w", b=2, w=W)

                if _is_direct(pi):
                    src = rv[:, 2 * sl:2 * sl + 2, 0:NF].rearrange(
                        "p b (h w) -> p b h w", w=W)
                    nc.vector.tensor_mul(pv4, src, window(pi))
                else:
                    we2 = we_pool.tile([128, 2, NF], BF16, tag="we")
                    nc.scalar.activation(we2[:], rv[:, 2 * sl:2 * sl + 2, 0:NF],
                                         AF.Copy, scale=1.0)
                    nc.vector.tensor_mul(
                        pv4, we2[:].rearrange("p b (h w) -> p b h w", w=W),
                        window(pi))

                if to_tree:
                    tree_j += 1
                    if tree_j % 8 == 0 or tree_j == n_tree:
                        nfull = 8 if tree_j % 8 == 0 else tree_j % 8
                        # chain the group's planes; the FINAL add lands in a
                        # dedicated "acc" tile (never alias tag "s": its slot
                        # would be held until the next group's chain, which
                        # deadlocks the pool)
                        s_ap = prod8[:, 0, :]
                        s_tile = None
                        for e in range(1, nfull):
                            tag = "acc" if (accT is None and e == nfull - 1) else "s"
                            d = acc_pool.tile([128, 2 * NF], BF16, tag=tag)
                            nc.vector.tensor_add(d[:], s_ap, prod8[:, e, :])
                            s_ap = d[:]
                            s_tile = d
                        if accT is None:
                            accT = s_tile
                        else:
                            na = acc_pool.tile([128, 2 * NF], BF16, tag="acc")
                            nc.vector.tensor_add(na[:], accT[:], s_ap)
                            accT = na
                else:
                    dve_planes.append(plane)

            flush_ident(1)
            # final identity-add carries stop=True to close both groups
            last = dve_planes.pop(0)
            for h in range(2):
                nc.tensor.matmul(accp[:, h * 512:h * 512 + NF], ident[:],
                                 last[:, ts(h, NF)], start=False,
                                 stop=True, skip_group_check=True)
            # evict the fp32 PE accumulator to bf16 SBUF
            nc.scalar.activation(
                accP_sb[:].rearrange("p (s x) -> p s x", x=NF),
                accp[:].rearrange("p (s x) -> p s x", x=512)[:, :, 0:NF],
                AF.Copy, scale=1.0)

        # merge ky-groups and the two accumulators, pipelined by halves:
        # inv[c] = accP[c]+accP[64+c]+accT[c]+accT[64+c], then +b2, ReLU
        tmpP = spool.tile([CMID, 2 * NF], BF16)
        tmpT = spool.tile([CMID, 2 * NF], BF16)
        m1 = spool.tile([CMID, 2 * NF], BF16)
        m2 = spool.tile([CMID, 2 * NF], BF16)
        m3 = spool.tile([CMID, 2 * NF], BF16)
        out2f = spool.tile([CMID, 2 * NF], BF16)
        for h in range(2):
            nc.sync.dma_start(out=tmpP[:, ts(h, NF)], in_=accP_sb[CMID:128, ts(h, NF)])
            nc.sync.dma_start(out=tmpT[:, ts(h, NF)], in_=accT[CMID:128, ts(h, NF)])
            nc.vector.tensor_add(m1[:, ts(h, NF)], accP_sb[0:CMID, ts(h, NF)],
                                 tmpP[:, ts(h, NF)])
            nc.vector.tensor_add(m2[:, ts(h, NF)], accT[0:CMID, ts(h, NF)],
                                 tmpT[:, ts(h, NF)])
            nc.vector.tensor_add(m3[:, ts(h, NF)], m1[:, ts(h, NF)],
                                 m2[:, ts(h, NF)])
            nc.scalar.activation(out2f[:, ts(h, NF)], m3[:, ts(h, NF)], AF.Relu,
                                 bias=vecs[0:CMID, 4:5], scale=1.0)

        # conv3 (g3 pre-folded into W3) + residual via identity matmul into
        # PSUM + bias b3 + relu
        with tc.tile_pool(name="p3", bufs=2, space="PSUM") as p3, \
             tc.tile_pool(name="ypool", bufs=2) as ypool:
            for nh in range(2):
                for mc in range(2):
                    ps = p3.tile([128, NF], F32, tag="ps3")
                    nc.tensor.matmul(ps[:], w3t[:, ts(mc, 128)], out2f[:, ts(nh, NF)],
                                     start=True, stop=False)
                    nc.tensor.matmul(ps[:], ident[:], xrb[:, mc, ts(nh, NF)],
                                     start=False, stop=True)
                    yr = ypool.tile([128, NF], F32, tag="yr")
                    nc.scalar.activation(yr[:], ps[:], AF.Relu,
                                         bias=vecs[:, 5 + mc:6 + mc], scale=1.0)
                    nc.sync.dma_start(
                        out=y_d[:].rearrange("(c p) n -> p c n", p=128)[:, mc, ts(nh, NF)],
                        in_=yr[:])

    nc.compile()
    names = dict(xb=xb_d.name, xr=xr_d.name, w1t=w1t_d.name, wrt=wrt_d.name,
                 wse=wse_d.name, w3t=w3t_d.name, vecs=vec_d.name,
                 ones=ones_d.name, ident=id_d.name, y=y_d.name)
    return nc, names


def _get_program():
    global _PROGRAM
    if _PROGRAM is None:
        _PROGRAM = _build_program()
    return _PROGRAM


def _bf16(a):
    return np.asarray(a, dtype=np.float32).astype(ml_dtypes.bfloat16)


def _host_inputs(x, W1, g1, b1, Wr, gr, br, Ws, bs, g2, b2, W3, g3, b3, names):
    x = np.asarray(x, dtype=np.float32)
    w1t = _bf16(np.asarray(W1).T)                      # [256, 64]
    wrt = _bf16(np.asarray(Wr).T)                      # [64, 16]
    # fold BN3 gamma into W3 (rows scaled per output channel)
    w3t = _bf16((np.asarray(g3)[:, None] * np.asarray(W3)).T)  # [64, 256]

    # span weights, 16x channel-expanded, tap-paired (ky, ky+7), bias row 16,
    # with BN2 gamma folded in per expanded channel.
    # wse[:, pi*128 + j]: j<64 -> tap (ky,kx), j>=64 -> tap (ky+7,kx) (zeros
    # for the ky=14 solo row).  pi = ky*15 + kx, ky in 0..7.
    Ws = np.asarray(Ws, dtype=np.float32)              # [900, 16]
    bs = np.asarray(bs, dtype=np.float32)              # [900]
    g2 = np.asarray(g2, dtype=np.float32)              # [64]
    gidx = np.arange(CMID) // GC                       # [64]
    WsT = Ws.reshape(G, K * K, RED)                    # [g, k, rho]
    bsr = bs.reshape(G, K * K)
    wse = np.zeros((RED + 1, NPAIR, 128), dtype=np.float32)
    for pi in range(NPAIR):
        if pi < 7 * K:
            ky, kx = pi // K, pi % K
            k1, k2 = ky * K + kx, (ky + 7) * K + kx
        else:
            kx = 2 * (pi - 7 * K)
            k1 = 14 * K + kx
            k2 = 14 * K + kx + 1 if kx + 1 < K else None
        wse[0:RED, pi, 0:CMID] = WsT[gidx, k1, :].T * g2[None, :]
        wse[RED, pi, 0:CMID] = bsr[gidx, k1] * g2
        if k2 is not None:
            wse[0:RED, pi, CMID:128] = WsT[gidx, k2, :].T * g2[None, :]
            wse[RED, pi, CMID:128] = bsr[gidx, k2] * g2
    wse4 = np.zeros((81, WCH * 128), dtype=np.float32)
    for pi in range(NPAIR):
        wc, wo = pi // WCH, pi % WCH
        wse4[32 * wc:32 * wc + RED + 1, wo * 128:(wo + 1) * 128] = wse[:, pi, :]
    wse = _bf16(wse4)

    vecs = np.zeros((128, 7), dtype=np.float32)
    vecs[0:CMID, 0] = g1
    vecs[0:CMID, 1] = b1
    vecs[0:RED, 2] = gr
    vecs[0:RED, 3] = br
    vecs[0:CMID, 4] = b2
    vecs[:, 5] = np.asarray(b3)[0:128]
    vecs[:, 6] = np.asarray(b3)[128:256]

    ident = np.eye(128, dtype=np.float32)

    in_maps = []
    core_geom = []
    for core in range(8):
        b = core // 4
        h0 = (core % 4) * HB
        xpad = np.zeros((CIN, HP, WP), dtype=np.float32)
        lo, hi = h0 - PAD, h0 + HB + PAD
        slo, shi = max(lo, 0), min(hi, H)
        xpad[:, slo - lo:shi - lo, PAD:PAD + W] = x[b, :, slo:shi, :]
        xbc = _bf16(xpad).reshape(CIN, NP)
        xrc = _bf16(np.ascontiguousarray(x[b, :, h0:h0 + HB, :]).reshape(COUT, HB * W))
        in_maps.append({
            names["xb"]: xbc,
            names["xr"]: xrc,
            names["w1t"]: w1t,
            names["wrt"]: wrt,
            names["wse"]: wse,
            names["w3t"]: w3t,
            names["vecs"]: vecs,
            names["ones"]: np.ones((1, 2 * NF), dtype=np.float32).astype(ml_dtypes.bfloat16),
            names["ident"]: _bf16(ident),
        })
        core_geom.append((b, h0))
    return in_maps, core_geom


def kernel(x, W1, g1, b1, Wr, gr, br, Ws, bs, g2, b2, W3, g3, b3,
           _want_results=False, _trace=False):
    nc, names = _get_program()
    in_maps, core_geom = _host_inputs(x, W1, g1, b1, Wr, gr, br, Ws, bs,
                                      g2, b2, W3, g3, b3, names)

    res = run_bass_kernel_spmd(nc, in_maps, list(range(8)), trace=_trace)

    y = np.empty((B, COUT, H, W), dtype=np.float32)
    for core, (b, h0) in enumerate(core_geom):
        y[b, :, h0:h0 + HB, :] = res.results[core][names["y"]].reshape(COUT, HB, W)
    if _want_results:
        return y, res
    return y


# revision 14
# speedup vs baseline: 1.6601x; 1.6601x over previous
"""Trainium2 Bass kernel for an involution Bottleneck block (B=2, Cin=256,
Cmid=64, Cout=256, H=W=56, K=15, G=4).

Sharding: 8 cores = 2 batches x 4 H-quarters (14 output rows each). Each core
receives a zero-padded input halo [256, 28, 70] (7 rows/cols each side), so no
inter-core communication is needed (halo compute is redundant).

Per-core pipeline (channels on SBUF partitions, pixels on free dim):
  conv1 1x1 (PE, bf16) -> BN+ReLU (ACT) -> out1 [64, 28x70] bf16, plus a copy
    shifted by +7 rows at partitions 64:128 and a +1-col copy (odd-kx align).
  reduce 1x1 (PE) -> BN+ReLU (ACT) -> r [16, 784] bf16 (+ ones row for bias).
  involution as 113 tap-PAIRS (ky, ky+7): per pair one span matmul with M=128
    (2 taps x 64 group-expanded channels, BN2 gamma folded in, bias via the
    ones-row at K=17) into a 6-bank PSUM ring (3 pairs in flight).  Products:
      E-pairs: ACT evicts the pair to bf16 SBUF, DVE tensor_mul with the
        shifted out1 window.
      D-pairs (~3/10): DVE multiplies directly from fp32 PSUM (no eviction),
        relieving ACT.
    Accumulation: most product planes are summed ON THE PE as fp32 via
    identity matmuls into a dedicated 2-bank PSUM accumulator (lagged a few
    pairs so the plane is ready); every 8th plane instead goes through a
    small DVE bf16 add tree to keep DVE/PE balanced.
  merge ky-halves + both accumulators (DMA + DVE adds) -> +b2, ReLU (ACT) ->
  conv3 1x1 with g3 folded into W3 (PE), accumulating the bf16 residual via
  an identity matmul -> +b3, ReLU (ACT) -> DMA out.
"""

import sys, types
sys.path.insert(0, "/opt/trn_rl_repo")

import numpy as np
import ml_dtypes
from contextlib import ExitStack

import concourse.bass as bass
import concourse.mybir as mybir
import concourse.tile as tile
from concourse import bacc
from concourse.bass import ts
from concourse.bass_utils import run_bass_kernel_spmd

BF16 = mybir.dt.bfloat16
F32 = mybir.dt.float32
AF = mybir.ActivationFunctionType

K = 15
G = 4
GC = 16
PAD = 7
CIN = 256
CMID = 64
RED = 16
COUT = 256
H = 56
W = 56
B = 2
HB = 14            # output rows per core
HP = HB + 2 * PAD  # 28 padded rows
WP = W + 2 * PAD   # 70 padded cols
NP = HP * WP       # 1960
HH = HB // 2       # 7 rows per half-block
NF = HH * W        # 392 pixels per half-block
NPAIR = 7 * K + 8  # 105 (ky,ky+7) pairs + 8 row-14 (kx,kx+1) pairs = 113
WCH = 38           # wse pairs per 32-partition chunk

_PROGRAM = None  # (nc, names) cache


def _build_program():
    nc = bacc.Bacc(None, target_bir_lowering=False, debug=False)
    with tile.TileContext(nc) as tc, ExitStack() as ctx:
        dram = ctx.enter_context(tc.tile_pool(name="dram", bufs=1, space="DRAM"))
        xb_d = dram.tile([CIN, NP], BF16, kind="ExternalInput", name="xb")
        xr_d = dram.tile([COUT, HB * W], BF16, kind="ExternalInput", name="xr")
        w1t_d = dram.tile([CIN, CMID], BF16, kind="ExternalInput", name="w1t")
        wrt_d = dram.tile([CMID, RED], BF16, kind="ExternalInput", name="wrt")
        wse_d = dram.tile([81, WCH * 128], BF16, kind="ExternalInput", name="wse")
        w3t_d = dram.tile([CMID, COUT], BF16, kind="ExternalInput", name="w3t")
        vec_d = dram.tile([128, 7], F32, kind="ExternalInput", name="vecs")
        ones_d = dram.tile([1, 2 * NF], BF16, kind="ExternalInput", name="ones")
        id_d = dram.tile([128, 128], BF16, kind="ExternalInput", name="ident")
        y_d = dram.tile([COUT, HB * W], F32, kind="ExternalOutput", name="y")

        wpool = ctx.enter_context(tc.tile_pool(name="weights", bufs=1))
        w1t = wpool.tile([128, 2, CMID], BF16)
        nc.sync.dma_start(out=w1t[:], in_=w1t_d[:].rearrange("(c p) m -> p c m", p=128))

        # x halo load in 4 chunks matching conv1's j-loop so conv1 can start
        # as soon as its first chunk lands
        xpool = ctx.enter_context(tc.tile_pool(name="xin", bufs=1))
        xb = xpool.tile([128, 2, NP], BF16)
        xb_src = xb_d[:].rearrange("(c p) n -> p c n", p=128)
        for j in (1, 2, 3, 0):
            nc.sync.dma_start(out=xb[:, :, ts(j, 490)], in_=xb_src[:, :, ts(j, 490)])

        wrt = wpool.tile([CMID, RED], BF16)
        nc.sync.dma_start(out=wrt[:], in_=wrt_d[:])
        vecs = wpool.tile([128, 7], F32)
        nc.sync.dma_start(out=vecs[:], in_=vec_d[:])
        ident = wpool.tile([128, 128], BF16)
        nc.sync.dma_start(out=ident[:], in_=id_d[:])
        # span weights in 3 vertical chunks at partitions 0/32/64 so the
        # DMA spans 81 partitions and fans out across the HWDGE queues
        wse = wpool.tile([81, WCH * 128], BF16)
        nc.sync.dma_start(out=wse[:], in_=wse_d[:])
        w3t = wpool.tile([CMID, COUT], BF16)
        nc.sync.dma_start(out=w3t[:], in_=w3t_d[:])
        xrb = xpool.tile([128, 2, HB * W], BF16)
        nc.sync.dma_start(out=xrb[:], in_=xr_d[:].rearrange("(c p) n -> p c n", p=128))

        opool = ctx.enter_context(tc.tile_pool(name="out1", bufs=1))
        out1p = opool.tile([128, NP], BF16)
        out1q = opool.tile([128, NP], BF16)
        out1r = opool.tile([128, NP], BF16)  # row-14 pairs: [0:64]=blockA, [64:128]=blockA<<1col

        # conv1: out1 = relu(g1 * (W1 @ x) + b1) over all 28x70 padded pixels
        with tc.tile_pool(name="p1", bufs=4, space="PSUM") as p1:
            for j in (1, 2, 3, 0):
                ps = p1.tile([CMID, 490], F32, tag="ps1")
                nc.tensor.matmul(ps[:], w1t[:, 0, :], xb[:, 0, ts(j, 490)],
                                 start=True, stop=False)
                nc.tensor.matmul(ps[:], w1t[:, 1, :], xb[:, 1, ts(j, 490)],
                                 start=False, stop=True)
                nc.scalar.activation(out1p[0:CMID, ts(j, 490)], ps[:], AF.Relu,
                                     bias=vecs[0:CMID, 1:2], scale=vecs[0:CMID, 0:1])

        # rows 7..27 duplicated at partitions 64:128 (the +7-row tap shift);
        # zero the unwritten tail there so zero-padded taps read 0, not junk.
        DUPW = (HP - HH) * WP  # 1470
        nc.vector.memset(out1p[CMID:128, DUPW:NP], 0.0)
        nc.vector.memset(out1q[CMID:128, DUPW - 1:NP], 0.0)
        nc.vector.memset(out1q[0:CMID, NP - 1:NP], 0.0)
        nc.sync.dma_start(out=out1p[CMID:128, 0:DUPW],
                          in_=out1p[0:CMID, HH * WP:NP])
        o3 = out1p[:].rearrange("p (h w) -> p h w", w=WP)
        o3q = out1q[:].rearrange("p (h w) -> p h w", w=WP)
        o3r = out1r[:].rearrange("p (h w) -> p h w", w=WP)

        spool = ctx.enter_context(tc.tile_pool(name="stage", bufs=1))
        r_sb = spool.tile([81, 2 * NF], BF16)
        # ones row (span bias via the K dimension); DMA since engines cannot
        # address a single partition at offset 16
        nc.sync.dma_start(out=r_sb[RED:RED + 1, :], in_=ones_d[:])

        # reduce: r = relu(gr * (Wr @ out1_central) + br), central 14x56 pixels
        with tc.tile_pool(name="pr", bufs=2, space="PSUM") as pr:
            for hhalf in range(2):
                ps = pr.tile([RED, NF], F32, tag="psr")
                nc.tensor.matmul(ps[:], wrt[:],
                                 o3[0:CMID, PAD + HH * hhalf:PAD + HH * (hhalf + 1), PAD:PAD + W],
                                 start=True, stop=True)
                nc.scalar.activation(r_sb[0:RED, ts(hhalf, NF)], ps[:], AF.Relu,
                                     bias=vecs[0:RED, 3:4], scale=vecs[0:RED, 2:3])

        # replicate r (+ones row) at partitions 32/64 to match the span
        # lhsT chunks' base partitions
        for c in range(1, 3):
            nc.sync.dma_start(out=r_sb[32 * c:32 * c + RED + 1, :],
                              in_=r_sb[0:RED + 1, :])

        # +1-col shifted copies (odd-kx 4B alignment) are same-partition, so
        # they run on DVE; placed after the reduce so r is produced first
        nc.vector.tensor_copy(out1q[0:CMID, 0:NP - 1], out1p[0:CMID, 1:NP])
        nc.vector.tensor_copy(out1q[CMID:128, 0:DUPW - 1], out1p[CMID:128, 1:DUPW])
        # row-14 pair source: partitions 64:128 hold block A shifted +1 col
        nc.vector.memset(out1r[CMID:128, NP - 1:NP], 0.0)
        nc.vector.tensor_copy(out1r[0:CMID, :], out1p[0:CMID, :])
        nc.sync.dma_start(out=out1r[CMID:128, 0:NP - 1], in_=out1p[0:CMID, 1:NP])

        def window(pi):
            """shifted out1 window for pair pi as a [128, 2, 7, 56] view"""
            if pi < 7 * K:
                ky, kx = pi // K, pi % K
                if kx % 2 == 0:
                    src_ = o3[:, ky:ky + 2 * HH, kx:kx + W]
                else:
                    src_ = o3q[:, ky:ky + 2 * HH, kx - 1:kx - 1 + W]
            else:
                kx = 2 * (pi - 7 * K)  # row-14 pair (14,kx)+(14,kx+1)
                src_ = o3r[:, 14:14 + 2 * HH, kx:kx + W]
            return src_.rearrange("p (b h) w -> p b h w", b=2)

        # involution span + multiply + accumulate.  ACT evicts quads, DVE
        # multiplies and tree-adds most planes; every 4th plane's
        # accumulation is offloaded to a gpsimd bf16 chain (accG) to shave
        # the DVE total.
        accT = None     # DVE tree accumulator (bf16)
        accG = None     # gpsimd chain accumulator (bf16)
        prod8 = None
        tree_j = 0
        n_chain = sum(1 for pi in range(NPAIR) if pi % 4 == 1)
        n_tree_planes = NPAIR - n_chain
        with tc.tile_pool(name="sp", bufs=1, space="PSUM") as sp, \
             tc.tile_pool(name="we", bufs=3) as we_pool, \
             tc.tile_pool(name="plp", bufs=4) as plane_pool, \
             tc.tile_pool(name="gacc", bufs=2) as g_pool, \
             tc.tile_pool(name="prod", bufs=2) as prod_pool, \
             tc.tile_pool(name="accp", bufs=2) as acc_pool:
            ring = sp.tile([128, 4096], F32)   # all 8 banks: 2 quads in flight
            rv = ring[:].rearrange("p (s x) -> p s x", x=512)

            for q in range(57):
                solo = 2 * q == NPAIR - 1
                pairs = [2 * q] if solo else [2 * q, 2 * q + 1]
                r0 = 4 * q % 8
                for idx, pi in enumerate(pairs):
                    wc, wo = pi // WCH, pi % WCH
                    lhsT = wse[32 * wc:32 * wc + RED + 1, ts(wo, 128)]
                    rr = r_sb[32 * wc:32 * wc + RED + 1, :]
                    for h in range(2):
                        sl = r0 + 2 * idx + h
                        nc.tensor.matmul(ring[:, sl * 512:sl * 512 + NF],
                                         lhsT, rr[:, ts(h, NF)],
                                         start=True, stop=True)
                nsl = 2 * len(pairs)
                we4 = we_pool.tile([128, nsl, NF], BF16,
                                   tag="we4" if nsl == 4 else "we2")
                nc.scalar.activation(we4[:], rv[:, r0:r0 + nsl, 0:NF], AF.Copy,
                                     scale=1.0)
                for idx, pi in enumerate(pairs):
                    wv = we4[:, 2 * idx:2 * idx + 2, :].rearrange(
                        "p b (h w) -> p b h w", w=W)
                    if pi % 4 == 1:  # gpsimd chain plane
                        pt = plane_pool.tile([128, 2 * NF], BF16, tag="plane")
                        nc.vector.tensor_mul(
                            pt[:].rearrange("p (b h w) -> p b h w", b=2, w=W),
                            wv, window(pi))
                        if accG is None:
                            accG = g_pool.tile([128, 2 * NF], BF16, tag="accG")
                            nc.gpsimd.tensor_copy(accG[:], pt[:])
                        else:
                            ng = g_pool.tile([128, 2 * NF], BF16, tag="accG")
                            nc.gpsimd.tensor_add(ng[:], accG[:], pt[:])
                            accG = ng
                        continue
                    if tree_j % 8 == 0:
                        prod8 = prod_pool.tile([128, 8, 2 * NF], BF16, tag="prod")
                    plane = prod8[:, tree_j % 8, :]
                    nc.vector.tensor_mul(
                        plane.rearrange("p (b h w) -> p b h w", b=2, w=W),
                        wv, window(pi))
                    tree_j += 1
                    if tree_j % 8 == 0 or tree_j == n_tree_planes:
                        nfull = 8 if tree_j % 8 == 0 else tree_j % 8
                        # chain the group's planes; the FINAL add lands in a
                        # dedicated "acc" tile (never alias tag "s": its slot
                        # would be held until the next group's chain, which
                        # deadlocks the pool)
                        s_ap = prod8[:, 0, :]
                        s_tile = None
                        for e in range(1, nfull):
                            tag = "acc" if (accT is None and e == nfull - 1) else "s"
                            d = acc_pool.tile([128, 2 * NF], BF16, tag=tag)
                            nc.vector.tensor_add(d[:], s_ap, prod8[:, e, :])
                            s_ap = d[:]
                            s_tile = d
                        if accT is None:
                            accT = s_tile
                        else:
                            na = acc_pool.tile([128, 2 * NF], BF16, tag="acc")
                            nc.vector.tensor_add(na[:], accT[:], s_ap)
                            accT = na

        # merge ky-groups and the two accumulators, pipelined by halves:
        # inv[c] = accP[c]+accP[64+c]+accT[c]+accT[64+c], then +b2, ReLU
        tmpP = spool.tile([CMID, 2 * NF], BF16)
        tmpT = spool.tile([CMID, 2 * NF], BF16)
        m1 = spool.tile([CMID, 2 * NF], BF16)
        m2 = spool.tile([CMID, 2 * NF], BF16)
        m3 = spool.tile([CMID, 2 * NF], BF16)
        out2f = spool.tile([CMID, 2 * NF], BF16)
        for h in range(2):
            nc.sync.dma_start(out=tmpP[:, ts(h, NF)], in_=accG[CMID:128, ts(h, NF)])
            nc.sync.dma_start(out=tmpT[:, ts(h, NF)], in_=accT[CMID:128, ts(h, NF)])
            nc.vector.tensor_add(m1[:, ts(h, NF)], accG[0:CMID, ts(h, NF)],
                                 tmpP[:, ts(h, NF)])
            nc.vector.tensor_add(m2[:, ts(h, NF)], accT[0:CMID, ts(h, NF)],
                                 tmpT[:, ts(h, NF)])
            nc.vector.tensor_add(m3[:, ts(h, NF)], m1[:, ts(h, NF)],
                                 m2[:, ts(h, NF)])
            nc.scalar.activation(out2f[:, ts(h, NF)], m3[:, ts(h, NF)], AF.Relu,
                                 bias=vecs[0:CMID, 4:5], scale=1.0)

        # conv3 (g3 pre-folded into W3) + residual via identity matmul into
        # PSUM + bias b3 + relu
        with tc.tile_pool(name="p3", bufs=2, space="PSUM") as p3, \
             tc.tile_pool(name="ypool", bufs=2) as ypool:
            for nh in range(2):
                for mc in range(2):
                    ps = p3.tile([128, NF], F32, tag="ps3")
                    nc.tensor.matmul(ps[:], w3t[:, ts(mc, 128)], out2f[:, ts(nh, NF)],
                                     start=True, stop=False)
                    nc.tensor.matmul(ps[:], ident[:], xrb[:, mc, ts(nh, NF)],
                                     start=False, stop=True)
                    yr = ypool.tile([128, NF], F32, tag="yr")
                    nc.scalar.activation(yr[:], ps[:], AF.Relu,
                                         bias=vecs[:, 5 + mc:6 + mc], scale=1.0)
                    nc.sync.dma_start(
                        out=y_d[:].rearrange("(c p) n -> p c n", p=128)[:, mc, ts(nh, NF)],
                        in_=yr[:])

    nc.compile()
    names = dict(xb=xb_d.name, xr=xr_d.name, w1t=w1t_d.name, wrt=wrt_d.name,
                 wse=wse_d.name, w3t=w3t_d.name, vecs=vec_d.name,
                 ones=ones_d.name, ident=id_d.name, y=y_d.name)
    return nc, names


def _get_program():
    global _PROGRAM
    if _PROGRAM is None:
        _PROGRAM = _build_program()
    return _PROGRAM


def _bf16(a):
    return np.asarray(a, dtype=np.float32).astype(ml_dtypes.bfloat16)


def _host_inputs(x, W1, g1, b1, Wr, gr, br, Ws, bs, g2, b2, W3, g3, b3, names):
    x = np.asarray(x, dtype=np.float32)
    w1t = _bf16(np.asarray(W1).T)                      # [256, 64]
    wrt = _bf16(np.asarray(Wr).T)                      # [64, 16]
    # fold BN3 gamma into W3 (rows scaled per output channel)
    w3t = _bf16((np.asarray(g3)[:, None] * np.asarray(W3)).T)  # [64, 256]

    # span weights, 16x channel-expanded, tap-paired (ky, ky+7), bias row 16,
    # with BN2 gamma folded in per expanded channel.
    # wse[:, pi*128 + j]: j<64 -> tap (ky,kx), j>=64 -> tap (ky+7,kx) (zeros
    # for the ky=14 solo row).  pi = ky*15 + kx, ky in 0..7.
    Ws = np.asarray(Ws, dtype=np.float32)              # [900, 16]
    bs = np.asarray(bs, dtype=np.float32)              # [900]
    g2 = np.asarray(g2, dtype=np.float32)              # [64]
    gidx = np.arange(CMID) // GC                       # [64]
    WsT = Ws.reshape(G, K * K, RED)                    # [g, k, rho]
    bsr = bs.reshape(G, K * K)
    wse = np.zeros((RED + 1, NPAIR, 128), dtype=np.float32)
    for pi in range(NPAIR):
        if pi < 7 * K:
            ky, kx = pi // K, pi % K
            k1, k2 = ky * K + kx, (ky + 7) * K + kx
        else:
            kx = 2 * (pi - 7 * K)
            k1 = 14 * K + kx
            k2 = 14 * K + kx + 1 if kx + 1 < K else None
        wse[0:RED, pi, 0:CMID] = WsT[gidx, k1, :].T * g2[None, :]
        wse[RED, pi, 0:CMID] = bsr[gidx, k1] * g2
        if k2 is not None:
            wse[0:RED, pi, CMID:128] = WsT[gidx, k2, :].T * g2[None, :]
            wse[RED, pi, CMID:128] = bsr[gidx, k2] * g2
    wse4 = np.zeros((81, WCH * 128), dtype=np.float32)
    for pi in range(NPAIR):
        wc, wo = pi // WCH, pi % WCH
        wse4[32 * wc:32 * wc + RED + 1, wo * 128:(wo + 1) * 128] = wse[:, pi, :]
    wse = _bf16(wse4)

    vecs = np.zeros((128, 7), dtype=np.float32)
    vecs[0:CMID, 0] = g1
    vecs[0:CMID, 1] = b1
    vecs[0:RED, 2] = gr
    vecs[0:RED, 3] = br
    vecs[0:CMID, 4] = b2
    vecs[:, 5] = np.asarray(b3)[0:128]
    vecs[:, 6] = np.asarray(b3)[128:256]

    ident = np.eye(128, dtype=np.float32)

    in_maps = []
    core_geom = []
    for core in range(8):
        b = core // 4
        h0 = (core % 4) * HB
        xpad = np.zeros((CIN, HP, WP), dtype=np.float32)
        lo, hi = h0 - PAD, h0 + HB + PAD
        slo, shi = max(lo, 0), min(hi, H)
        xpad[:, slo - lo:shi - lo, PAD:PAD + W] = x[b, :, slo:shi, :]
        xbc = _bf16(xpad).reshape(CIN, NP)
        xrc = _bf16(np.ascontiguousarray(x[b, :, h0:h0 + HB, :]).reshape(COUT, HB * W))
        in_maps.append({
            names["xb"]: xbc,
            names["xr"]: xrc,
            names["w1t"]: w1t,
            names["wrt"]: wrt,
            names["wse"]: wse,
            names["w3t"]: w3t,
            names["vecs"]: vecs,
            names["ones"]: np.ones((1, 2 * NF), dtype=np.float32).astype(ml_dtypes.bfloat16),
            names["ident"]: _bf16(ident),
        })
        core_geom.append((b, h0))
    return in_maps, core_geom


def kernel(x, W1, g1, b1, Wr, gr, br, Ws, bs, g2, b2, W3, g3, b3,
           _want_results=False, _trace=False):
    nc, names = _get_program()
    in_maps, core_geom = _host_inputs(x, W1, g1, b1, Wr, gr, br, Ws, bs,
                                      g2, b2, W3, g3, b3, names)

    res = run_bass_kernel_spmd(nc, in_maps, list(range(8)), trace=_trace)

    y = np.empty((B, COUT, H, W), dtype=np.float32)
    for core, (b, h0) in enumerate(core_geom):
        y[b, :, h0:h0 + HB, :] = res.results[core][names["y"]].reshape(COUT, HB, W)
    if _want_results:
        return y, res
    return y


# revision 17
# speedup vs baseline: 1.9853x; 1.1959x over previous
"""Trainium2 Bass kernel for an involution Bottleneck block (B=2, Cin=256,
Cmid=64, Cout=256, H=W=56, K=15, G=4).

Sharding: 8 cores = 2 batches x 4 H-quarters (14 output rows each). Each core
receives a zero-padded input halo [256, 28, 70] (7 rows/cols each side), so no
inter-core communication is needed (halo compute is redundant).

Per-core pipeline (channels on SBUF partitions, pixels on free dim):
  conv1 1x1 (PE, bf16) -> BN+ReLU (ACT) -> out1 [64, 28x70] bf16, plus a copy
    shifted by +7 rows at partitions 64:128 and a +1-col copy (odd-kx align).
  reduce 1x1 (PE) -> BN+ReLU (ACT) -> r [16, 784] bf16 (+ ones row for bias).
  involution as 113 tap-PAIRS (ky, ky+7): per pair one span matmul with M=128
    (2 taps x 64 group-expanded channels, BN2 gamma folded in, bias via the
    ones-row at K=17) into a 6-bank PSUM ring (3 pairs in flight).  Products:
      E-pairs: ACT evicts the pair to bf16 SBUF, DVE tensor_mul with the
        shifted out1 window.
      D-pairs (~3/10): DVE multiplies directly from fp32 PSUM (no eviction),
        relieving ACT.
    Accumulation: most product planes are summed ON THE PE as fp32 via
    identity matmuls into a dedicated 2-bank PSUM accumulator (lagged a few
    pairs so the plane is ready); every 8th plane instead goes through a
    small DVE bf16 add tree to keep DVE/PE balanced.
  merge ky-halves + both accumulators (DMA + DVE adds) -> +b2, ReLU (ACT) ->
  conv3 1x1 with g3 folded into W3 (PE), accumulating the bf16 residual via
  an identity matmul -> +b3, ReLU (ACT) -> DMA out.
"""

import sys, types
sys.path.insert(0, "/opt/trn_rl_repo")

import numpy as np
import ml_dtypes
from contextlib import ExitStack

import concourse.bass as bass
import concourse.mybir as mybir
import concourse.tile as tile
from concourse import bacc
from concourse.bass import ts
from concourse.bass_utils import run_bass_kernel_spmd

BF16 = mybir.dt.bfloat16
F32 = mybir.dt.float32
AF = mybir.ActivationFunctionType

K = 15
G = 4
GC = 16
PAD = 7
CIN = 256
CMID = 64
RED = 16
COUT = 256
H = 56
W = 56
B = 2
HB = 14            # output rows per core
HP = HB + 2 * PAD  # 28 padded rows
WP = W + 2 * PAD   # 70 padded cols
NP = HP * WP       # 1960
HH = HB // 2       # 7 rows per half-block
NF = HH * W        # 392 pixels per half-block
NPAIR = 7 * K + 8  # 105 (ky,ky+7) pairs + 8 row-14 (kx,kx+1) pairs = 113
WCH = 38           # wse pairs per 32-partition chunk

_PROGRAM = None  # (nc, names) cache


def _build_program():
    nc = bacc.Bacc(None, target_bir_lowering=False, debug=False)
    with tile.TileContext(nc) as tc, ExitStack() as ctx:
        dram = ctx.enter_context(tc.tile_pool(name="dram", bufs=1, space="DRAM"))
        xb_d = dram.tile([CIN, NP], BF16, kind="ExternalInput", name="xb")
        xr_d = dram.tile([COUT, HB * W], BF16, kind="ExternalInput", name="xr")
        w1t_d = dram.tile([CIN, CMID], BF16, kind="ExternalInput", name="w1t")
        wrt_d = dram.tile([CMID, RED], BF16, kind="ExternalInput", name="wrt")
        wse_d = dram.tile([81, WCH * 128], BF16, kind="ExternalInput", name="wse")
        w3t_d = dram.tile([CMID, COUT], BF16, kind="ExternalInput", name="w3t")
        vec_d = dram.tile([128, 7], F32, kind="ExternalInput", name="vecs")
        ones_d = dram.tile([1, 2 * NF], BF16, kind="ExternalInput", name="ones")
        id_d = dram.tile([128, 128], BF16, kind="ExternalInput", name="ident")
        y_d = dram.tile([COUT, HB * W], F32, kind="ExternalOutput", name="y")

        wpool = ctx.enter_context(tc.tile_pool(name="weights", bufs=1))
        w1t = wpool.tile([128, 2, CMID], BF16)
        nc.sync.dma_start(out=w1t[:], in_=w1t_d[:].rearrange("(c p) m -> p c m", p=128))

        # x halo load in 4 chunks matching conv1's j-loop so conv1 can start
        # as soon as its first chunk lands
        xpool = ctx.enter_context(tc.tile_pool(name="xin", bufs=1))
        xb = xpool.tile([128, 2, NP], BF16)
        xb_src = xb_d[:].rearrange("(c p) n -> p c n", p=128)
        for j in (1, 2, 3, 0):
            nc.sync.dma_start(out=xb[:, :, ts(j, 490)], in_=xb_src[:, :, ts(j, 490)])

        wrt = wpool.tile([CMID, RED], BF16)
        nc.sync.dma_start(out=wrt[:], in_=wrt_d[:])
        vecs = wpool.tile([128, 7], F32)
        nc.sync.dma_start(out=vecs[:], in_=vec_d[:])
        ident = wpool.tile([128, 128], BF16)
        nc.sync.dma_start(out=ident[:], in_=id_d[:])
        # span weights in 3 vertical chunks at partitions 0/32/64 so the
        # DMA spans 81 partitions and fans out across the HWDGE queues
        wse = wpool.tile([81, WCH * 128], BF16)
        nc.sync.dma_start(out=wse[:], in_=wse_d[:])
        w3t = wpool.tile([CMID, COUT], BF16)
        nc.sync.dma_start(out=w3t[:], in_=w3t_d[:])
        xrb = xpool.tile([128, 2, HB * W], BF16)
        nc.sync.dma_start(out=xrb[:], in_=xr_d[:].rearrange("(c p) n -> p c n", p=128))

        opool = ctx.enter_context(tc.tile_pool(name="out1", bufs=1))
        out1p = opool.tile([128, NP], BF16)
        out1q = opool.tile([128, NP], BF16)
        out1r = opool.tile([128, NP], BF16)  # row-14 pairs: [0:64]=blockA, [64:128]=blockA<<1col

        # conv1: out1 = relu(g1 * (W1 @ x) + b1) over all 28x70 padded pixels
        with tc.tile_pool(name="p1", bufs=4, space="PSUM") as p1:
            for j in (1, 2, 3, 0):
                ps = p1.tile([CMID, 490], F32, tag="ps1")
                nc.tensor.matmul(ps[:], w1t[:, 0, :], xb[:, 0, ts(j, 490)],
                                 start=True, stop=False)
                nc.tensor.matmul(ps[:], w1t[:, 1, :], xb[:, 1, ts(j, 490)],
                                 start=False, stop=True)
                nc.scalar.activation(out1p[0:CMID, ts(j, 490)], ps[:], AF.Relu,
                                     bias=vecs[0:CMID, 1:2], scale=vecs[0:CMID, 0:1])

        # rows 7..27 duplicated at partitions 64:128 (the +7-row tap shift);
        # zero the unwritten tail there so zero-padded taps read 0, not junk.
        DUPW = (HP - HH) * WP  # 1470
        nc.vector.memset(out1p[CMID:128, DUPW:NP], 0.0)
        nc.vector.memset(out1q[CMID:128, DUPW - 1:NP], 0.0)
        nc.vector.memset(out1q[0:CMID, NP - 1:NP], 0.0)
        nc.sync.dma_start(out=out1p[CMID:128, 0:DUPW],
                          in_=out1p[0:CMID, HH * WP:NP])
        o3 = out1p[:].rearrange("p (h w) -> p h w", w=WP)
        o3q = out1q[:].rearrange("p (h w) -> p h w", w=WP)
        o3r = out1r[:].rearrange("p (h w) -> p h w", w=WP)

        spool = ctx.enter_context(tc.tile_pool(name="stage", bufs=1))
        r_sb = spool.tile([81, 2 * NF], BF16)
        # ones row (span bias via the K dimension); DMA since engines cannot
        # address a single partition at offset 16
        nc.sync.dma_start(out=r_sb[RED:RED + 1, :], in_=ones_d[:])

        # reduce: r = relu(gr * (Wr @ out1_central) + br), central 14x56 pixels
        with tc.tile_pool(name="pr", bufs=2, space="PSUM") as pr:
            for hhalf in range(2):
                ps = pr.tile([RED, NF], F32, tag="psr")
                nc.tensor.matmul(ps[:], wrt[:],
                                 o3[0:CMID, PAD + HH * hhalf:PAD + HH * (hhalf + 1), PAD:PAD + W],
                                 start=True, stop=True)
                nc.scalar.activation(r_sb[0:RED, ts(hhalf, NF)], ps[:], AF.Relu,
                                     bias=vecs[0:RED, 3:4], scale=vecs[0:RED, 2:3])

        # replicate r (+ones row) at partitions 32/64 to match the span
        # lhsT chunks' base partitions
        for c in range(1, 3):
            nc.sync.dma_start(out=r_sb[32 * c:32 * c + RED + 1, :],
                              in_=r_sb[0:RED + 1, :])

        # +1-col shifted copies (odd-kx 4B alignment) are same-partition, so
        # they run on DVE; placed after the reduce so r is produced first
        nc.vector.tensor_copy(out1q[0:CMID, 0:NP - 1], out1p[0:CMID, 1:NP])
        nc.vector.tensor_copy(out1q[CMID:128, 0:DUPW - 1], out1p[CMID:128, 1:DUPW])
        # row-14 pair source: partitions 64:128 hold block A shifted +1 col
        nc.vector.memset(out1r[CMID:128, NP - 1:NP], 0.0)
        nc.vector.tensor_copy(out1r[0:CMID, :], out1p[0:CMID, :])
        nc.sync.dma_start(out=out1r[CMID:128, 0:NP - 1], in_=out1p[0:CMID, 1:NP])

        def window(pi):
            """shifted out1 window for pair pi as a [128, 2, 7, 56] view"""
            if pi < 7 * K:
                ky, kx = pi // K, pi % K
                if kx % 2 == 0:
                    src_ = o3[:, ky:ky + 2 * HH, kx:kx + W]
                else:
                    src_ = o3q[:, ky:ky + 2 * HH, kx - 1:kx - 1 + W]
            else:
                kx = 2 * (pi - 7 * K)  # row-14 pair (14,kx)+(14,kx+1)
                src_ = o3r[:, 14:14 + 2 * HH, kx:kx + W]
            return src_.rearrange("p (b h) w -> p b h w", b=2)

        # involution span + multiply + accumulate.  ACT evicts quads; DVE
        # multiplies and reduces all planes (gpsimd is useless here: its
        # SBUF streams run 3.5us/plane AND degrade DVE throughput ~40%).
        accT = None     # DVE tree accumulator (bf16)
        prod8 = None
        tree_j = 0
        n_tree_planes = NPAIR
        with tc.tile_pool(name="sp", bufs=1, space="PSUM") as sp, \
             tc.tile_pool(name="we", bufs=3) as we_pool, \
             tc.tile_pool(name="prod", bufs=2) as prod_pool, \
             tc.tile_pool(name="accp", bufs=2) as acc_pool:
            ring = sp.tile([128, 4096], F32)   # all 8 banks: 2 quads in flight
            rv = ring[:].rearrange("p (s x) -> p s x", x=512)

            for q in range(57):
                solo = 2 * q == NPAIR - 1
                pairs = [2 * q] if solo else [2 * q, 2 * q + 1]
                r0 = 4 * q % 8
                for idx, pi in enumerate(pairs):
                    wc, wo = pi // WCH, pi % WCH
                    lhsT = wse[32 * wc:32 * wc + RED + 1, ts(wo, 128)]
                    rr = r_sb[32 * wc:32 * wc + RED + 1, :]
                    for h in range(2):
                        sl = r0 + 2 * idx + h
                        nc.tensor.matmul(ring[:, sl * 512:sl * 512 + NF],
                                         lhsT, rr[:, ts(h, NF)],
                                         start=True, stop=True)
                nsl = 2 * len(pairs)
                we4 = we_pool.tile([128, nsl, NF], BF16,
                                   tag="we4" if nsl == 4 else "we2")
                nc.scalar.activation(we4[:], rv[:, r0:r0 + nsl, 0:NF], AF.Copy,
                                     scale=1.0)
                for idx, pi in enumerate(pairs):
                    wv = we4[:, 2 * idx:2 * idx + 2, :].rearrange(
                        "p b (h w) -> p b h w", w=W)
                    if tree_j % 8 == 0:
                        prod8 = prod_pool.tile([128, 8, 2 * NF], BF16, tag="prod")
                    plane = prod8[:, tree_j % 8, :]
                    nc.vector.tensor_mul(
                        plane.rearrange("p (b h w) -> p b h w", b=2, w=W),
                        wv, window(pi))
                    tree_j += 1
                    if tree_j % 8 == 0 or tree_j == n_tree_planes:
                        nfull = 8 if tree_j % 8 == 0 else tree_j % 8
                        # chain the group's planes; the FINAL add lands in a
                        # dedicated "acc" tile (never alias tag "s": its slot
                        # would be held until the next group's chain, which
                        # deadlocks the pool)
                        s_ap = prod8[:, 0, :]
                        s_tile = None
                        for e in range(1, nfull):
                            tag = "acc" if (accT is None and e == nfull - 1) else "s"
                            d = acc_pool.tile([128, 2 * NF], BF16, tag=tag)
                            nc.vector.tensor_add(d[:], s_ap, prod8[:, e, :])
                            s_ap = d[:]
                            s_tile = d
                        if accT is None:
                            accT = s_tile
                        else:
                            na = acc_pool.tile([128, 2 * NF], BF16, tag="acc")
                            nc.vector.tensor_add(na[:], accT[:], s_ap)
                            accT = na

        # merge ky-groups, pipelined by halves:
        # inv[c] = accT[c] + accT[64+c], then +b2, ReLU
        tmpT = spool.tile([CMID, 2 * NF], BF16)
        m1 = spool.tile([CMID, 2 * NF], BF16)
        out2f = spool.tile([CMID, 2 * NF], BF16)
        for h in range(2):
            nc.sync.dma_start(out=tmpT[:, ts(h, NF)], in_=accT[CMID:128, ts(h, NF)])
            nc.vector.tensor_add(m1[:, ts(h, NF)], accT[0:CMID, ts(h, NF)],
                                 tmpT[:, ts(h, NF)])
            nc.scalar.activation(out2f[:, ts(h, NF)], m1[:, ts(h, NF)], AF.Relu,
                                 bias=vecs[0:CMID, 4:5], scale=1.0)

        # conv3 (g3 pre-folded into W3) + residual via identity matmul into
        # PSUM + bias b3 + relu
        with tc.tile_pool(name="p3", bufs=2, space="PSUM") as p3, \
             tc.tile_pool(name="ypool", bufs=2) as ypool:
            for nh in range(2):
                for mc in range(2):
                    ps = p3.tile([128, NF], F32, tag="ps3")
                    nc.tensor.matmul(ps[:], w3t[:, ts(mc, 128)], out2f[:, ts(nh, NF)],
                                     start=True, stop=False)
                    nc.tensor.matmul(ps[:], ident[:], xrb[:, mc, ts(nh, NF)],
                                     start=False, stop=True)
                    yr = ypool.tile([128, NF], F32, tag="yr")
                    nc.scalar.activation(yr[:], ps[:], AF.Relu,
                                         bias=vecs[:, 5 + mc:6 + mc], scale=1.0)
                    nc.sync.dma_start(
                        out=y_d[:].rearrange("(c p) n -> p c n", p=128)[:, mc, ts(nh, NF)],
                        in_=yr[:])

    nc.compile()
    names = dict(xb=xb_d.name, xr=xr_d.name, w1t=w1t_d.name, wrt=wrt_d.name,
                 wse=wse_d.name, w3t=w3t_d.name, vecs=vec_d.name,
                 ones=ones_d.name, ident=id_d.name, y=y_d.name)
    return nc, names


def _get_program():
    global _PROGRAM
    if _PROGRAM is None:
        _PROGRAM = _build_program()
    return _PROGRAM


def _bf16(a):
    return np.asarray(a, dtype=np.float32).astype(ml_dtypes.bfloat16)


def _host_inputs(x, W1, g1, b1, Wr, gr, br, Ws, bs, g2, b2, W3, g3, b3, names):
    x = np.asarray(x, dtype=np.float32)
    w1t = _bf16(np.asarray(W1).T)                      # [256, 64]
    wrt = _bf16(np.asarray(Wr).T)                      # [64, 16]
    # fold BN3 gamma into W3 (rows scaled per output channel)
    w3t = _bf16((np.asarray(g3)[:, None] * np.asarray(W3)).T)  # [64, 256]

    # span weights, 16x channel-expanded, tap-paired (ky, ky+7), bias row 16,
    # with BN2 gamma folded in per expanded channel.
    # wse[:, pi*128 + j]: j<64 -> tap (ky,kx), j>=64 -> tap (ky+7,kx) (zeros
    # for the ky=14 solo row).  pi = ky*15 + kx, ky in 0..7.
    Ws = np.asarray(Ws, dtype=np.float32)              # [900, 16]
    bs = np.asarray(bs, dtype=np.float32)              # [900]
    g2 = np.asarray(g2, dtype=np.float32)              # [64]
    gidx = np.arange(CMID) // GC                       # [64]
    WsT = Ws.reshape(G, K * K, RED)                    # [g, k, rho]
    bsr = bs.reshape(G, K * K)
    wse = np.zeros((RED + 1, NPAIR, 128), dtype=np.float32)
    for pi in range(NPAIR):
        if pi < 7 * K:
            ky, kx = pi // K, pi % K
            k1, k2 = ky * K + kx, (ky + 7) * K + kx
        else:
            kx = 2 * (pi - 7 * K)
            k1 = 14 * K + kx
            k2 = 14 * K + kx + 1 if kx + 1 < K else None
        wse[0:RED, pi, 0:CMID] = WsT[gidx, k1, :].T * g2[None, :]
        wse[RED, pi, 0:CMID] = bsr[gidx, k1] * g2
        if k2 is not None:
            wse[0:RED, pi, CMID:128] = WsT[gidx, k2, :].T * g2[None, :]
            wse[RED, pi, CMID:128] = bsr[gidx, k2] * g2
    wse4 = np.zeros((81, WCH * 128), dtype=np.float32)
    for pi in range(NPAIR):
        wc, wo = pi // WCH, pi % WCH
        wse4[32 * wc:32 * wc + RED + 1, wo * 128:(wo + 1) * 128] = wse[:, pi, :]
    wse = _bf16(wse4)

    vecs = np.zeros((128, 7), dtype=np.float32)
    vecs[0:CMID, 0] = g1
    vecs[0:CMID, 1] = b1
    vecs[0:RED, 2] = gr
    vecs[0:RED, 3] = br
    vecs[0:CMID, 4] = b2
    vecs[:, 5] = np.asarray(b3)[0:128]
    vecs[:, 6] = np.asarray(b3)[128:256]

    ident = np.eye(128, dtype=np.float32)

    in_maps = []
    core_geom = []
    for core in range(8):
        b = core // 4
        h0 = (core % 4) * HB
        xpad = np.zeros((CIN, HP, WP), dtype=np.float32)
        lo, hi = h0 - PAD, h0 + HB + PAD
        slo, shi = max(lo, 0), min(hi, H)
        xpad[:, slo - lo:shi - lo, PAD:PAD + W] = x[b, :, slo:shi, :]
        xbc = _bf16(xpad).reshape(CIN, NP)
        xrc = _bf16(np.ascontiguousarray(x[b, :, h0:h0 + HB, :]).reshape(COUT, HB * W))
        in_maps.append({
            names["xb"]: xbc,
            names["xr"]: xrc,
            names["w1t"]: w1t,
            names["wrt"]: wrt,
            names["wse"]: wse,
            names["w3t"]: w3t,
            names["vecs"]: vecs,
            names["ones"]: np.ones((1, 2 * NF), dtype=np.float32).astype(ml_dtypes.bfloat16),
            names["ident"]: _bf16(ident),
        })
        core_geom.append((b, h0))
    return in_maps, core_geom


def kernel(x, W1, g1, b1, Wr, gr, br, Ws, bs, g2, b2, W3, g3, b3,
           _want_results=False, _trace=False):
    nc, names = _get_program()
    in_maps, core_geom = _host_inputs(x, W1, g1, b1, Wr, gr, br, Ws, bs,
                                      g2, b2, W3, g3, b3, names)

    res = run_bass_kernel_spmd(nc, in_maps, list(range(8)), trace=_trace)

    y = np.empty((B, COUT, H, W), dtype=np.float32)
    for core, (b, h0) in enumerate(core_geom):
        y[b, :, h0:h0 + HB, :] = res.results[core][names["y"]].reshape(COUT, HB, W)
    if _want_results:
        return y, res
    return y
